# revision 5
# baseline (speedup 1.0000x reference)
"""MoE block (B=4,T=2048,D=2048,E=4,H=8192,TOPK=2,cap=2048) on 8 TRN2 NeuronCores.

Strategy:
  - Router + top-k + capacity selection on host (exact jax-on-CPU replication of
    the reference routing math, so routing decisions match bit-for-bit).
  - Expert-parallel device FFN: core c handles expert c//2, token half c%2.
    Each core computes yT = W2[e]^T @ gelu(W1[e]^T @ xT) for its 1024 tokens.
    All matmuls in bf16 (same 1 col/cycle PE rate as fp32r but half the DMA,
    FWL-accelerated LDWEIGHTS, and no early HBM starvation), accumulation and
    y output in f32.
  - Host combines: scale by router prob and scatter-add into the output.

Device kernel layout (per core; host pre-transposes so every DMA line is
contiguous per partition):
  xT   [128, NT, KC, TT]   xT[p, t, kc, m]   = tok[t*TT+m, kc*128+p]   bf16
  w1a  [128, 64, 16, 128]  w1a[p, hc, kc, h] = W1[e][kc*128+p, hc*128+h] bf16
  w2s  [128, 16, 64, 128]  w2s[p, dc, hc, d] = W2[e][hc*128+p, dc*128+d] bf16
  yT   [128, 16, 1024]     yT[p, dc, m]      = y[m, dc*128+p]          f32
H is processed in 8 passes of 1024 (8 H-chunks of 128) so each weight byte
streams from HBM exactly once (DMA ~68MB in /16MB out per core).
Per pass: phase A computes hT (bf16) for all 1024 tokens — pass 0 t-outer
(so the second token-tile's x DMA gets ~28us of slack), later passes pair
the two token tiles per kc into two concurrent PSUM groups; phase B (same
pairing) emits a bf16 partial y_q per pass straight to DRAM via an ACT
PSUM->SBUF copy (ACT is idle in phase B; no DVE accumulation chain), and
the host sums the 8 partials in f32 during combine.
Matmuls are [128,128]x[128,512] bf16 at the 216ns/MM streaming floor
(512 cols / 2.4GHz + NX issue; LDWEIGHTS 97ns via FWL, fully hidden).
Measured ~905us/core HW exec (fp32r baseline: 961us) = 97.4% PE-active;
the rest is fixed NEFF init (~7.2us), DMA-bound rampup (~4us), periodic
432ns PE issue hiccups (~5us, hardware), and the epilogue (~5.5us).
Note: the chip sometimes sits in the P0 power state (PE 2.4->2.0GHz), which
inflates any measurement by ~1.2x; gaps then pace at 259ns instead of 216.
"""

import os

import numpy as np
import ml_dtypes

BF16 = ml_dtypes.bfloat16

B, T, D, E, H = 4, 2048, 2048, 4, 8192
TOPK = 2
N = B * T
CAP = N // E          # 2048 tokens per expert
M = CAP // 2          # 1024 tokens per core
KC = D // 128         # 16
HC = H // 128         # 64
TT = 512              # token tile
NT = M // TT          # 2 token tiles
NQ_G = 8              # H passes; device emits one bf16 partial-y per pass

_nc_cache = [None]


def _build_nc():
    import concourse.tile as tile
    import concourse.mybir as mybir
    from concourse import bacc
    from concourse.bass import ts

    F32 = mybir.dt.float32
    BF = mybir.dt.bfloat16
    GELU = mybir.ActivationFunctionType.Gelu
    COPY = mybir.ActivationFunctionType.Copy

    nc = bacc.Bacc(None, target_bir_lowering=False)
    xT = nc.declare_dram_parameter("xT", [128, NT, KC, TT], BF, isOutput=False)
    w1a = nc.declare_dram_parameter("w1a", [128, HC, KC, 128], BF, isOutput=False)
    w2s = nc.declare_dram_parameter("w2s", [128, KC, HC, 128], BF, isOutput=False)
    yT = nc.declare_dram_parameter("yT", [128, NQ_G, KC, M], BF, isOutput=True)

    HQ = 8  # H-chunks (of 128) per pass
    NQ = HC // HQ  # 8 passes; weights stream exactly once
    assert NQ == NQ_G

    with tile.TileContext(nc) as tc:
        with (
            tc.tile_pool(name="xpool", bufs=1) as xpool,
            tc.tile_pool(name="ystage", bufs=6) as ystage,
            tc.tile_pool(name="w1pool", bufs=10) as w1pool,
            tc.tile_pool(name="w2pool", bufs=3) as w2pool,
            tc.tile_pool(name="hpool", bufs=1) as hpool,
            tc.tile_pool(name="psa", bufs=4, space="PSUM") as psa,
            tc.tile_pool(name="psb", bufs=4, space="PSUM") as psb,
        ):
            x_sb = xpool.tile([128, NT, KC, TT], BF, tag="x")
            # Warm the PE HAM clock (cold 1.2GHz -> warm 2.4GHz needs ~3.4us of
            # sustained PE activity) with junk matmuls on a zeroed tile while
            # the startup DMAs are still in flight.
            # Small junk matmuls start the PE busy-window (HAM 1.2->2.4GHz
            # warmup needs ~3.4us) as early as possible: a [128,128] tile
            # memsets in ~150ns (vs 522ns for 512 cols), and N=128 MMs
            # (~107ns cold each) give fine queue granularity so the first
            # real MM dequeues right when its data lands.
            # The first x/w1 chunks only land ~5.6us after init-exit (early
            # DMA is far below peak rate), so the junk-MM bridge must span
            # that whole window or the HAM MID-window (~3.4us idle) drops the
            # PE back to 1.2GHz and the first real MMs run cold (measured
            # 634ns vs 379). 52 x 107ns cold N=128 MMs cover it with fine
            # dequeue granularity.
            # 46 MMs x 107ns cold = ~4.9us, ending right at the measured
            # first-data arrival (~12.6us) — enough to hold the HAM busy
            # window (a sub-3.4us idle gap cannot re-throttle), short enough
            # not to queue ahead of the first real MM.
            warm_sb = xpool.tile([128, 128], BF, tag="warm")
            nc.gpsimd.memset(warm_sb[:], 0.0)
            ps_warm = psa.tile([64, 128], F32, tag="psa")
            fill_state = [0]

            def fillers(n):
                # Junk MMs on a zeroed tile: keep the PE busy (HAM warm
                # grant needs a ~full busy window or it re-throttles to
                # 1.2GHz) while DMA-paced. ~107ns each cold, 54ns warm.
                for k in range(n):
                    nc.tensor.matmul(
                        ps_warm[:],
                        lhsT=warm_sb[:, :64],
                        rhs=warm_sb[:],
                        start=(k == 0),
                        stop=(k == n - 1),
                    )
                fill_state[0] += 1

            fillers(26)
            # Startup DMA order v3. Measured: the two HW-DGE queues (SP +
            # ACT) share one global early-bandwidth ramp (~25 -> 350 B/ns
            # by ~13.5us), so dual queues don't add bandwidth — but
            # splitting the critical set across both halves each queue's
            # backlog so the critical 2.5MB (x[t0] + w1[0]) completes
            # ~16.5us instead of ~20.5us. Interleave by need-order:
            #   SP : x[kc0] x[kc1] x[kc4:8] x[kc12:16]  then x[t1]
            #   ACT: w1[0]q0 x[kc2:4] w1[0]q1 x[kc8:12] w1[0]q2 w1[0]q3
            #        then w1[1..7] whole
            N_EARLY_W1 = 8
            w1_early = [
                w1pool.tile([128, KC, 128], BF, tag="w1", name="w1_early0")
            ]
            nc.scalar.dma_start(w1_early[0][:, ts(0, 4), :], w1a[:, 0, ts(0, 4)])
            nc.sync.dma_start(x_sb[:, 0, 0:1, :], xT[:, 0, 0:1, :])
            nc.sync.dma_start(x_sb[:, 0, 1:2, :], xT[:, 0, 1:2, :])
            nc.scalar.dma_start(x_sb[:, 0, 2:4, :], xT[:, 0, 2:4, :])
            nc.sync.dma_start(x_sb[:, 0, 4:8, :], xT[:, 0, 4:8, :])
            nc.scalar.dma_start(w1_early[0][:, ts(1, 4), :], w1a[:, 0, ts(1, 4)])
            nc.scalar.dma_start(x_sb[:, 0, 8:12, :], xT[:, 0, 8:12, :])
            nc.sync.dma_start(x_sb[:, 0, 12:16, :], xT[:, 0, 12:16, :])
            nc.scalar.dma_start(w1_early[0][:, ts(2, 4), :], w1a[:, 0, ts(2, 4)])
            nc.scalar.dma_start(w1_early[0][:, ts(3, 4), :], w1a[:, 0, ts(3, 4)])
            for i in range(1, N_EARLY_W1):
                w1_sb = w1pool.tile([128, KC, 128], BF, tag="w1", name=f"w1_early{i}")
                nc.scalar.dma_start(w1_sb[:], w1a[:, i])
                w1_early.append(w1_sb)
            for c in range(2):
                nc.sync.dma_start(
                    x_sb[:, 1, ts(c, 8), :], xT[:, 1, ts(c, 8), :]
                )

            for q in range(NQ):
                hT_sb = hpool.tile([128, HQ, M], BF, tag="h")

                # Phase A: hT[q] = gelu(W1[:, q]^T @ x), all M tokens.
                # Pass 0 runs t-outer (all t0 groups, then t1) so x[t1] has a
                # half-phase (~28us) of DMA slack. Later passes pair the two
                # token tiles per kc (kc-outer, t-inner into two concurrent
                # PSUM groups) so consecutive MMs share the stationary weights.
                if q == 0:
                    w1_tiles = []
                    for t in range(NT):
                        for i in range(HQ):
                            if t == 0:
                                if i < N_EARLY_W1:
                                    w1_sb = w1_early[i]
                                else:
                                    w1_sb = w1pool.tile(
                                        [128, KC, 128], BF, tag="w1"
                                    )
                                    nc.sync.dma_start(w1_sb[:], w1a[:, i])
                                w1_tiles.append(w1_sb)
                            else:
                                w1_sb = w1_tiles[i]
                            ps = psa.tile([128, TT], F32, tag="psa")
                            # Group 0 trails the x[t0] DMA stream; filler
                            # bursts absorb the predicted inter-chunk waits
                            # so the PE never idles (sized from the traced
                            # arrival curve, cold-rate 107ns each).
                            fill_after = (
                                {1: 8, 3: 16, 7: 14, 11: 8, 15: 8}
                                if (t == 0 and i == 0)
                                else {}
                            )
                            for kc in range(KC):
                                nc.tensor.matmul(
                                    ps[:],
                                    lhsT=w1_sb[:, kc],
                                    rhs=x_sb[:, t, kc, :],
                                    start=(kc == 0),
                                    stop=(kc == KC - 1),
                                )
                                if kc in fill_after:
                                    fillers(fill_after[kc])
                            nc.scalar.activation(
                                hT_sb[:, i, ts(t, TT)], ps[:], GELU
                            )
                else:
                    for i in range(HQ):
                        hc = q * HQ + i
                        w1_sb = w1pool.tile([128, KC, 128], BF, tag="w1")
                        nc.sync.dma_start(w1_sb[:], w1a[:, hc])
                        psp = [
                            psa.tile([128, TT], F32, tag="psa", name=f"psA{q}_{i}_{t}")
                            for t in range(NT)
                        ]
                        for kc in range(KC):
                            for t in range(NT):
                                nc.tensor.matmul(
                                    psp[t][:],
                                    lhsT=w1_sb[:, kc],
                                    rhs=x_sb[:, t, kc, :],
                                    start=(kc == 0),
                                    stop=(kc == KC - 1),
                                )
                        for t in range(NT):
                            nc.scalar.activation(
                                hT_sb[:, i, ts(t, TT)], psp[t][:], GELU
                            )

                # Phase B: emit partial y_q = W2[q]^T @ hT[q] (bf16) straight
                # to DRAM; the host sums the NQ partials in f32. ACT does the
                # PSUM->SBUF copy (it is idle during phase B), freeing each
                # PSUM bank ~427ns after its group ends — no DVE add chain,
                # no bank-recycle stalls.
                for dc in range(KC):
                    w2_sb = w2pool.tile([128, HQ, 128], BF, tag="w2")
                    nc.sync.dma_start(w2_sb[:], w2s[:, dc, ts(q, HQ)])
                    psp2 = [
                        psb.tile([128, TT], F32, tag="psb", name=f"psB{q}_{dc}_{t}")
                        for t in range(NT)
                    ]
                    for i in range(HQ):
                        for t in range(NT):
                            nc.tensor.matmul(
                                psp2[t][:],
                                lhsT=w2_sb[:, i],
                                rhs=hT_sb[:, i, ts(t, TT)],
                                start=(i == 0),
                                stop=(i == HQ - 1),
                            )
                    for t in range(NT):
                        yst = ystage.tile([128, TT], BF, tag="yst")
                        nc.scalar.activation(yst[:], psp2[t][:], COPY)
                        nc.sync.dma_start(yT[:, q, dc, ts(t, TT)], yst[:])
    nc.finalize()
    return nc


def _route(x, Wg, bg):
    """Replicate the reference routing math with jax on CPU.

    Returns (sel_idx, p): [E, CAP] int64 token ids and [E, CAP] f32 weights.
    """
    import jax
    import jax.numpy as jnp

    cpu = jax.devices("cpu")[0]
    with jax.default_device(cpu):
        flat_x = jnp.asarray(x.reshape(N, D))
        logits = flat_x @ jnp.asarray(Wg) + jnp.asarray(bg)
        top_vals, top_idx = jax.lax.top_k(logits, TOPK)
        sparse = jnp.full_like(logits, -jnp.inf)
        sparse = sparse.at[jnp.arange(N)[:, None], top_idx].set(top_vals)
        probs = jax.nn.softmax(sparse, axis=-1)

        sel_idx = np.zeros((E, CAP), dtype=np.int64)
        p_all = np.zeros((E, CAP), dtype=np.float32)
        for i in range(E):
            assigned = (top_idx == i).any(axis=-1)
            score = jnp.where(assigned, probs[:, i], -jnp.inf)
            sel_p, sidx = jax.lax.top_k(score, CAP)
            p = jnp.where(jnp.isfinite(sel_p), sel_p, 0.0)
            sel_idx[i] = np.asarray(sidx)
            p_all[i] = np.asarray(p)
    return sel_idx, p_all


def kernel(x, Wg, bg, W1, W2):
    from concourse.bass_utils import run_bass_kernel_spmd

    x = np.asarray(x, dtype=np.float32)
    W1 = np.asarray(W1, dtype=np.float32)
    W2 = np.asarray(W2, dtype=np.float32)
    sel_idx, p_all = _route(x, np.asarray(Wg, np.float32), np.asarray(bg, np.float32))

    flat_x = x.reshape(N, D)

    # Host dispatch + weight shuffles (bf16).
    w1a = [
        np.ascontiguousarray(
            W1[e].reshape(KC, 128, HC, 128).transpose(1, 2, 0, 3)
        ).astype(BF16)
        for e in range(E)
    ]
    w2s = [
        np.ascontiguousarray(
            W2[e].reshape(HC, 128, KC, 128).transpose(1, 2, 0, 3)
        ).astype(BF16)
        for e in range(E)
    ]
    in_maps = []
    for c in range(8):
        e, half = divmod(c, 2)
        tok = flat_x[sel_idx[e, half * M : (half + 1) * M]]  # [M, D]
        # xT[p, t, kc, m] = tok[t*TT+m, kc*128+p]
        xT = np.ascontiguousarray(
            tok.reshape(NT, TT, KC, 128).transpose(3, 0, 2, 1)
        ).astype(BF16)
        in_maps.append({"xT": xT, "w1a": w1a[e], "w2s": w2s[e]})

    if _nc_cache[0] is None:
        _nc_cache[0] = _build_nc()
    nc = _nc_cache[0]

    trace = bool(os.environ.get("BASS_MOE_TRACE"))
    kwargs = {}
    if trace:
        import sys
        import types

        try:
            from antenv.axon_hooks import get_axon_ntff_profile_hook  # noqa: F401
        except ImportError:
            from trn_agent_boot.trn_boot import _ntff_profile_via_ctypes

            hook = _ntff_profile_via_ctypes("/opt/axon/libaxon_pjrt.so")
            mod = types.ModuleType("antenv.axon_hooks")
            mod.get_axon_ntff_profile_hook = lambda: hook
            import antenv  # noqa: F401

            sys.modules["antenv.axon_hooks"] = mod
        tcores = [int(c) for c in os.environ.get("BASS_MOE_TRACE_CORES", "0").split(",")]
        kwargs = {"trace": True, "trace_cores": tcores}

    res = run_bass_kernel_spmd(nc, in_maps, core_ids=list(range(8)), **kwargs)
    if trace:
        kernel.last_exec_time_ns = res.exec_time_ns
        if res.exec_time_ns is not None:
            print(f"HW exec time: {res.exec_time_ns} ns")

    # Host combine: y = yT^T * p, scatter-add per expert in order.
    out = np.zeros((N, D), dtype=np.float32)
    for c in range(8):
        e, half = divmod(c, 2)
        yq = np.asarray(res.results[c]["yT"], dtype=np.float32)  # [128, NQ, KC, M]
        yT = yq.sum(axis=1)  # [128, KC, M]
        y = yT.transpose(1, 0, 2).reshape(D, M).T  # [M, D]
        p = p_all[e, half * M : (half + 1) * M]
        np.add.at(out, sel_idx[e, half * M : (half + 1) * M], y * p[:, None])
    return out.reshape(B, T, D)



# revision 10
# speedup vs baseline: 1.0203x; 1.0203x over previous
"""MoE block (B=4,T=2048,D=2048,E=4,H=8192,TOPK=2,cap=2048) on 8 TRN2 NeuronCores.

Strategy:
  - Router + top-k + capacity selection on host (exact jax-on-CPU replication of
    the reference routing math, so routing decisions match bit-for-bit).
  - Expert-parallel device FFN: core c handles expert c//2, token half c%2.
    Each core computes yT = W2[e]^T @ gelu(W1[e]^T @ xT) for its 1024 tokens.
    All matmuls in bf16 (same 1 col/cycle PE rate as fp32r but half the DMA,
    FWL-accelerated LDWEIGHTS, and no early HBM starvation), accumulation and
    y output in f32.
  - Host combines: scale by router prob and scatter-add into the output.

Device kernel layout (per core; host pre-transposes so every DMA line is
contiguous per partition):
  xT   [128, NT, KC, TT]   xT[p, t, kc, m]   = tok[t*TT+m, kc*128+p]   bf16
  w1a  [128, 64, 16, 128]  w1a[p, hc, kc, h] = W1[e][kc*128+p, hc*128+h] bf16
  w2s  [128, 16, 64, 128]  w2s[p, dc, hc, d] = W2[e][hc*128+p, dc*128+d] bf16
  yT   [128, 16, 1024]     yT[p, dc, m]      = y[m, dc*128+p]          f32
H is processed in 8 passes of 1024 (8 H-chunks of 128) so each weight byte
streams from HBM exactly once (DMA ~68MB in /16MB out per core).
Per pass: phase A computes hT (bf16) for all 1024 tokens — pass 0 t-outer
(so the second token-tile's x DMA gets ~28us of slack), later passes pair
the two token tiles per kc into two concurrent PSUM groups; phase B (same
pairing) emits a bf16 partial y_q per pass straight to DRAM via an ACT
PSUM->SBUF copy (ACT is idle in phase B; no DVE accumulation chain), and
the host sums the 8 partials in f32 during combine.
Matmuls are [128,128]x[128,512] bf16 at the 216ns/MM streaming floor
(512 cols / 2.4GHz + NX issue; LDWEIGHTS 97ns via FWL, fully hidden).
Measured ~905us/core HW exec (fp32r baseline: 961us) = 97.4% PE-active;
the rest is fixed NEFF init (~7.2us), DMA-bound rampup (~4us), periodic
432ns PE issue hiccups (~5us, hardware), and the epilogue (~5.5us).
Note: the chip sometimes sits in the P0 power state (PE 2.4->2.0GHz), which
inflates any measurement by ~1.2x; gaps then pace at 259ns instead of 216.
"""

import os

import numpy as np
import ml_dtypes

BF16 = ml_dtypes.bfloat16

B, T, D, E, H = 4, 2048, 2048, 4, 8192
TOPK = 2
N = B * T
CAP = N // E          # 2048 tokens per expert
M = CAP // 2          # 1024 tokens per core
KC = D // 128         # 16
HC = H // 128         # 64
TT = 512              # token tile
NT = M // TT          # 2 token tiles
NQ_G = 8              # H passes; device emits one bf16 partial-y per pass
FP8_PASS = 6          # this pass's phase B runs in fp8 DoubleRow (2x pump)
S2_SCALE = 128.0      # pow2 prescale on the fp8 W2 slice (undone in ACT copy)

_nc_cache = [None]


def _build_nc():
    import concourse.tile as tile
    import concourse.mybir as mybir
    from concourse import bacc
    from concourse.bass import ts

    F32 = mybir.dt.float32
    BF = mybir.dt.bfloat16
    F8 = mybir.dt.float8e4
    DR = mybir.MatmulPerfMode.DoubleRow
    GELU = mybir.ActivationFunctionType.Gelu
    COPY = mybir.ActivationFunctionType.Copy

    nc = bacc.Bacc(None, target_bir_lowering=False)
    xT = nc.declare_dram_parameter("xT", [128, NT, KC, TT], BF, isOutput=False)
    w1a = nc.declare_dram_parameter("w1a", [128, HC, KC, 128], BF, isOutput=False)
    w2s = nc.declare_dram_parameter("w2s", [128, KC, HC, 128], BF, isOutput=False)
    # fp8 pair-interleaved W2 slice for the FP8_PASS H-chunk (x S2_SCALE):
    # w2f8[p, dc, u, j, d] = W2[e][(FP8_PASS*8 + 2u+j)*128 + p, dc*128+d]*S2
    w2f8 = nc.declare_dram_parameter("w2f8", [128, KC, 4, 2, 128], F8, isOutput=False)
    yT = nc.declare_dram_parameter("yT", [128, NQ_G, KC, M], BF, isOutput=True)

    HQ = 8  # H-chunks (of 128) per pass
    NQ = HC // HQ  # 8 passes; weights stream exactly once
    assert NQ == NQ_G

    with tile.TileContext(nc) as tc:
        with (
            tc.tile_pool(name="xpool", bufs=1) as xpool,
            tc.tile_pool(name="ystage", bufs=6) as ystage,
            tc.tile_pool(name="w1pool", bufs=10) as w1pool,
            tc.tile_pool(name="w2pool", bufs=3) as w2pool,
            tc.tile_pool(name="hpool", bufs=1) as hpool,
            tc.tile_pool(name="psa", bufs=4, space="PSUM") as psa,
            tc.tile_pool(name="psb", bufs=4, space="PSUM") as psb,
        ):
            x_sb = xpool.tile([128, NT, KC, TT], BF, tag="x")
            # Warm the PE HAM clock (cold 1.2GHz -> warm 2.4GHz needs ~3.4us of
            # sustained PE activity) with junk matmuls on a zeroed tile while
            # the startup DMAs are still in flight.
            # Small junk matmuls start the PE busy-window (HAM 1.2->2.4GHz
            # warmup needs ~3.4us) as early as possible: a [128,128] tile
            # memsets in ~150ns (vs 522ns for 512 cols), and N=128 MMs
            # (~107ns cold each) give fine queue granularity so the first
            # real MM dequeues right when its data lands.
            # The first x/w1 chunks only land ~5.6us after init-exit (early
            # DMA is far below peak rate), so the junk-MM bridge must span
            # that whole window or the HAM MID-window (~3.4us idle) drops the
            # PE back to 1.2GHz and the first real MMs run cold (measured
            # 634ns vs 379). 52 x 107ns cold N=128 MMs cover it with fine
            # dequeue granularity.
            # 46 MMs x 107ns cold = ~4.9us, ending right at the measured
            # first-data arrival (~12.6us) — enough to hold the HAM busy
            # window (a sub-3.4us idle gap cannot re-throttle), short enough
            # not to queue ahead of the first real MM.
            warm_sb = xpool.tile([128, 128], BF, tag="warm")
            nc.gpsimd.memset(warm_sb[:], 0.0)
            ps_warm = psa.tile([64, 128], F32, tag="psa")
            fill_state = [0]

            def fillers(n):
                # Junk MMs on a zeroed tile: keep the PE busy (HAM warm
                # grant needs a ~full busy window or it re-throttles to
                # 1.2GHz) while DMA-paced. ~107ns each cold, 54ns warm.
                for k in range(n):
                    nc.tensor.matmul(
                        ps_warm[:],
                        lhsT=warm_sb[:, :64],
                        rhs=warm_sb[:],
                        start=(k == 0),
                        stop=(k == n - 1),
                    )
                fill_state[0] += 1

            fillers(26)
            # Startup DMA order v3. Measured: the two HW-DGE queues (SP +
            # ACT) share one global early-bandwidth ramp (~25 -> 350 B/ns
            # by ~13.5us), so dual queues don't add bandwidth — but
            # splitting the critical set across both halves each queue's
            # backlog so the critical 2.5MB (x[t0] + w1[0]) completes
            # ~16.5us instead of ~20.5us. Interleave by need-order:
            #   SP : x[kc0] x[kc1] x[kc4:8] x[kc12:16]  then x[t1]
            #   ACT: w1[0]q0 x[kc2:4] w1[0]q1 x[kc8:12] w1[0]q2 w1[0]q3
            #        then w1[1..7] whole
            N_EARLY_W1 = 8
            w1_early = [
                w1pool.tile([128, KC, 128], BF, tag="w1", name="w1_early0")
            ]
            nc.scalar.dma_start(w1_early[0][:, ts(0, 4), :], w1a[:, 0, ts(0, 4)])
            nc.sync.dma_start(x_sb[:, 0, 0:1, :], xT[:, 0, 0:1, :])
            nc.sync.dma_start(x_sb[:, 0, 1:2, :], xT[:, 0, 1:2, :])
            nc.scalar.dma_start(x_sb[:, 0, 2:4, :], xT[:, 0, 2:4, :])
            nc.sync.dma_start(x_sb[:, 0, 4:8, :], xT[:, 0, 4:8, :])
            nc.scalar.dma_start(w1_early[0][:, ts(1, 4), :], w1a[:, 0, ts(1, 4)])
            nc.scalar.dma_start(x_sb[:, 0, 8:12, :], xT[:, 0, 8:12, :])
            nc.sync.dma_start(x_sb[:, 0, 12:16, :], xT[:, 0, 12:16, :])
            nc.scalar.dma_start(w1_early[0][:, ts(2, 4), :], w1a[:, 0, ts(2, 4)])
            nc.scalar.dma_start(w1_early[0][:, ts(3, 4), :], w1a[:, 0, ts(3, 4)])
            for i in range(1, N_EARLY_W1):
                w1_sb = w1pool.tile([128, KC, 128], BF, tag="w1", name=f"w1_early{i}")
                nc.scalar.dma_start(w1_sb[:], w1a[:, i])
                w1_early.append(w1_sb)
            for c in range(2):
                nc.sync.dma_start(
                    x_sb[:, 1, ts(c, 8), :], xT[:, 1, ts(c, 8), :]
                )

            for q in range(NQ):
                # Pass FP8_PASS keeps h in e4m3: its phase B runs as fp8
                # DoubleRow matmuls (pairs of adjacent h-chunks), which
                # stream 2 MACs/cell/cycle. Quantizing h + W2 for 1/8 of
                # H adds ~1.3e-2 L2 error (measured on the real seed)
                # on top of the 3.8e-3 bf16 baseline — inside the 2e-2
                # budget. gelu's ACT writes convert PSUM f32 -> fp8.
                hT_sb = hpool.tile(
                    [128, HQ, M], F8 if q == FP8_PASS else BF, tag="h"
                )

                # Phase A: hT[q] = gelu(W1[:, q]^T @ x), all M tokens.
                # Pass 0 runs t-outer (all t0 groups, then t1) so x[t1] has a
                # half-phase (~28us) of DMA slack. Later passes pair the two
                # token tiles per kc (kc-outer, t-inner into two concurrent
                # PSUM groups) so consecutive MMs share the stationary weights.
                if q == 0:
                    w1_tiles = []
                    for t in range(NT):
                        for i in range(HQ):
                            if t == 0:
                                if i < N_EARLY_W1:
                                    w1_sb = w1_early[i]
                                else:
                                    w1_sb = w1pool.tile(
                                        [128, KC, 128], BF, tag="w1"
                                    )
                                    nc.sync.dma_start(w1_sb[:], w1a[:, i])
                                w1_tiles.append(w1_sb)
                            else:
                                w1_sb = w1_tiles[i]
                            ps = psa.tile([128, TT], F32, tag="psa")
                            # Group 0 trails the x[t0] DMA stream; filler
                            # bursts absorb the predicted inter-chunk waits
                            # so the PE never idles (sized from the traced
                            # arrival curve, cold-rate 107ns each).
                            fill_after = (
                                {1: 8, 3: 16, 7: 14, 11: 8, 15: 8}
                                if (t == 0 and i == 0)
                                else {}
                            )
                            for kc in range(KC):
                                nc.tensor.matmul(
                                    ps[:],
                                    lhsT=w1_sb[:, kc],
                                    rhs=x_sb[:, t, kc, :],
                                    start=(kc == 0),
                                    stop=(kc == KC - 1),
                                )
                                if kc in fill_after:
                                    fillers(fill_after[kc])
                            nc.scalar.activation(
                                hT_sb[:, i, ts(t, TT)], ps[:], GELU
                            )
                else:
                    for i in range(HQ):
                        hc = q * HQ + i
                        w1_sb = w1pool.tile([128, KC, 128], BF, tag="w1")
                        nc.sync.dma_start(w1_sb[:], w1a[:, hc])
                        psp = [
                            psa.tile([128, TT], F32, tag="psa", name=f"psA{q}_{i}_{t}")
                            for t in range(NT)
                        ]
                        for kc in range(KC):
                            for t in range(NT):
                                nc.tensor.matmul(
                                    psp[t][:],
                                    lhsT=w1_sb[:, kc],
                                    rhs=x_sb[:, t, kc, :],
                                    start=(kc == 0),
                                    stop=(kc == KC - 1),
                                )
                        for t in range(NT):
                            nc.scalar.activation(
                                hT_sb[:, i, ts(t, TT)], psp[t][:], GELU
                            )

                # Phase B: emit partial y_q = W2[q]^T @ hT[q] (bf16) straight
                # to DRAM; the host sums the NQ partials in f32. ACT does the
                # PSUM->SBUF copy (it is idle during phase B), freeing each
                # PSUM bank ~427ns after its group ends — no DVE add chain,
                # no bank-recycle stalls.
                for dc in range(KC):
                    psp2 = [
                        psb.tile([128, TT], F32, tag="psb", name=f"psB{q}_{dc}_{t}")
                        for t in range(NT)
                    ]
                    if q == FP8_PASS:
                        w28_sb = w2pool.tile([128, 4, 2, 128], F8, tag="w2")
                        nc.sync.dma_start(w28_sb[:], w2f8[:, dc])
                        for u in range(HQ // 2):
                            for t in range(NT):
                                nc.tensor.matmul(
                                    psp2[t][:],
                                    lhsT=w28_sb[:, u],
                                    rhs=hT_sb[:, 2 * u : 2 * u + 2, ts(t, TT)],
                                    start=(u == 0),
                                    stop=(u == HQ // 2 - 1),
                                    perf_mode=DR,
                                )
                        for t in range(NT):
                            yst = ystage.tile([128, TT], BF, tag="yst")
                            nc.scalar.activation(
                                yst[:], psp2[t][:], COPY, scale=1.0 / S2_SCALE
                            )
                            nc.sync.dma_start(yT[:, q, dc, ts(t, TT)], yst[:])
                    else:
                        w2_sb = w2pool.tile([128, HQ, 128], BF, tag="w2")
                        nc.sync.dma_start(w2_sb[:], w2s[:, dc, ts(q, HQ)])
                        for i in range(HQ):
                            for t in range(NT):
                                nc.tensor.matmul(
                                    psp2[t][:],
                                    lhsT=w2_sb[:, i],
                                    rhs=hT_sb[:, i, ts(t, TT)],
                                    start=(i == 0),
                                    stop=(i == HQ - 1),
                                )
                        for t in range(NT):
                            yst = ystage.tile([128, TT], BF, tag="yst")
                            nc.scalar.activation(yst[:], psp2[t][:], COPY)
                            nc.sync.dma_start(yT[:, q, dc, ts(t, TT)], yst[:])
    nc.finalize()
    return nc


def _route(x, Wg, bg):
    """Replicate the reference routing math with jax on CPU.

    Returns (sel_idx, p): [E, CAP] int64 token ids and [E, CAP] f32 weights.
    """
    import jax
    import jax.numpy as jnp

    cpu = jax.devices("cpu")[0]
    with jax.default_device(cpu):
        flat_x = jnp.asarray(x.reshape(N, D))
        logits = flat_x @ jnp.asarray(Wg) + jnp.asarray(bg)
        top_vals, top_idx = jax.lax.top_k(logits, TOPK)
        sparse = jnp.full_like(logits, -jnp.inf)
        sparse = sparse.at[jnp.arange(N)[:, None], top_idx].set(top_vals)
        probs = jax.nn.softmax(sparse, axis=-1)

        sel_idx = np.zeros((E, CAP), dtype=np.int64)
        p_all = np.zeros((E, CAP), dtype=np.float32)
        for i in range(E):
            assigned = (top_idx == i).any(axis=-1)
            score = jnp.where(assigned, probs[:, i], -jnp.inf)
            sel_p, sidx = jax.lax.top_k(score, CAP)
            p = jnp.where(jnp.isfinite(sel_p), sel_p, 0.0)
            sel_idx[i] = np.asarray(sidx)
            p_all[i] = np.asarray(p)
    return sel_idx, p_all


def kernel(x, Wg, bg, W1, W2):
    from concourse.bass_utils import run_bass_kernel_spmd

    x = np.asarray(x, dtype=np.float32)
    W1 = np.asarray(W1, dtype=np.float32)
    W2 = np.asarray(W2, dtype=np.float32)
    sel_idx, p_all = _route(x, np.asarray(Wg, np.float32), np.asarray(bg, np.float32))

    flat_x = x.reshape(N, D)

    # Host dispatch + weight shuffles (bf16).
    w1a = [
        np.ascontiguousarray(
            W1[e].reshape(KC, 128, HC, 128).transpose(1, 2, 0, 3)
        ).astype(BF16)
        for e in range(E)
    ]
    w2s = [
        np.ascontiguousarray(
            W2[e].reshape(HC, 128, KC, 128).transpose(1, 2, 0, 3)
        ).astype(BF16)
        for e in range(E)
    ]
    H0 = FP8_PASS * 8 * 128
    w2f8 = [
        np.ascontiguousarray(
            (W2[e][H0 : H0 + 1024] * S2_SCALE)
            .reshape(4, 2, 128, KC, 128)
            .transpose(2, 3, 0, 1, 4)
        ).astype(ml_dtypes.float8_e4m3)
        for e in range(E)
    ]
    in_maps = []
    for c in range(8):
        e, half = divmod(c, 2)
        tok = flat_x[sel_idx[e, half * M : (half + 1) * M]]  # [M, D]
        # xT[p, t, kc, m] = tok[t*TT+m, kc*128+p]
        xT = np.ascontiguousarray(
            tok.reshape(NT, TT, KC, 128).transpose(3, 0, 2, 1)
        ).astype(BF16)
        in_maps.append({"xT": xT, "w1a": w1a[e], "w2s": w2s[e], "w2f8": w2f8[e]})

    if _nc_cache[0] is None:
        _nc_cache[0] = _build_nc()
    nc = _nc_cache[0]

    trace = bool(os.environ.get("BASS_MOE_TRACE"))
    kwargs = {}
    if trace:
        import sys
        import types

        try:
            from antenv.axon_hooks import get_axon_ntff_profile_hook  # noqa: F401
        except ImportError:
            from trn_agent_boot.trn_boot import _ntff_profile_via_ctypes

            hook = _ntff_profile_via_ctypes("/opt/axon/libaxon_pjrt.so")
            mod = types.ModuleType("antenv.axon_hooks")
            mod.get_axon_ntff_profile_hook = lambda: hook
            import antenv  # noqa: F401

            sys.modules["antenv.axon_hooks"] = mod
        tcores = [int(c) for c in os.environ.get("BASS_MOE_TRACE_CORES", "0").split(",")]
        kwargs = {"trace": True, "trace_cores": tcores}

    res = run_bass_kernel_spmd(nc, in_maps, core_ids=list(range(8)), **kwargs)
    if trace:
        kernel.last_exec_time_ns = res.exec_time_ns
        if res.exec_time_ns is not None:
            print(f"HW exec time: {res.exec_time_ns} ns")

    # Host combine: y = yT^T * p, scatter-add per expert in order.
    out = np.zeros((N, D), dtype=np.float32)
    for c in range(8):
        e, half = divmod(c, 2)
        yq = np.asarray(res.results[c]["yT"], dtype=np.float32)  # [128, NQ, KC, M]
        yT = yq.sum(axis=1)  # [128, KC, M]
        y = yT.transpose(1, 0, 2).reshape(D, M).T  # [M, D]
        p = p_all[e, half * M : (half + 1) * M]
        np.add.at(out, sel_idx[e, half * M : (half + 1) * M], y * p[:, None])
    return out.reshape(B, T, D)



# revision 17
# speedup vs baseline: 1.0427x; 1.0220x over previous
"""MoE block (B=4,T=2048,D=2048,E=4,H=8192,TOPK=2,cap=2048) on 8 TRN2 NeuronCores.

Strategy:
  - Router + top-k + capacity selection on host (exact jax-on-CPU replication of
    the reference routing math, so routing decisions match bit-for-bit).
  - Expert-parallel device FFN: core c handles expert c//2, token half c%2.
    Each core computes yT = W2[e]^T @ gelu(W1[e]^T @ xT) for its 1024 tokens.
    All matmuls in bf16 (same 1 col/cycle PE rate as fp32r but half the DMA,
    FWL-accelerated LDWEIGHTS, and no early HBM starvation), accumulation and
    y output in f32.
  - Host combines: scale by router prob and scatter-add into the output.

Device kernel layout (per core; host pre-transposes so every DMA line is
contiguous per partition):
  xT   [128, NT, KC, TT]   xT[p, t, kc, m]   = tok[t*TT+m, kc*128+p]   bf16
  w1a  [128, 64, 16, 128]  w1a[p, hc, kc, h] = W1[e][kc*128+p, hc*128+h] bf16
  w2s  [128, 16, 64, 128]  w2s[p, dc, hc, d] = W2[e][hc*128+p, dc*128+d] bf16
  yT   [128, 16, 1024]     yT[p, dc, m]      = y[m, dc*128+p]          f32
H is processed in 8 passes of 1024 (8 H-chunks of 128) so each weight byte
streams from HBM exactly once (DMA ~68MB in /16MB out per core).
Per pass: phase A computes hT (bf16) for all 1024 tokens — pass 0 t-outer
(so the second token-tile's x DMA gets ~28us of slack), later passes pair
the two token tiles per kc into two concurrent PSUM groups; phase B (same
pairing) emits a bf16 partial y_q per pass straight to DRAM via an ACT
PSUM->SBUF copy (ACT is idle in phase B; no DVE accumulation chain), and
the host sums the 8 partials in f32 during combine.
Matmuls are [128,128]x[128,512] bf16 at the 216ns/MM streaming floor
(512 cols / 2.4GHz + NX issue; LDWEIGHTS 97ns via FWL, fully hidden).
Measured ~905us/core HW exec (fp32r baseline: 961us) = 97.4% PE-active;
the rest is fixed NEFF init (~7.2us), DMA-bound rampup (~4us), periodic
432ns PE issue hiccups (~5us, hardware), and the epilogue (~5.5us).
Note: the chip sometimes sits in the P0 power state (PE 2.4->2.0GHz), which
inflates any measurement by ~1.2x; gaps then pace at 259ns instead of 216.
"""

import os

import numpy as np
import ml_dtypes

BF16 = ml_dtypes.bfloat16

B, T, D, E, H = 4, 2048, 2048, 4, 8192
TOPK = 2
N = B * T
CAP = N // E          # 2048 tokens per expert
M = CAP // 2          # 1024 tokens per core
KC = D // 128         # 16
HC = H // 128         # 64
TT = 512              # token tile
NT = M // TT          # 2 token tiles
NQ_G = 8              # H passes; device emits one bf16 partial-y per pass
FP8_PASS = 6          # this pass's phase B runs in fp8 DoubleRow (2x pump)
FP8H_PASS = 5         # this pass's phase B is fp8 for h-chunks 0..3 only
S2_SCALE = 128.0      # pow2 prescale on ALL of W2 (undone in the ACT copy);
                      # makes the fp8 slices (needs the scale to clear e4m3
                      # subnormals) and the bf16 rest uniformly scaled so
                      # mixed bf16+fp8 PSUM groups stay consistent

_nc_cache = [None]


def _build_nc():
    import concourse.tile as tile
    import concourse.mybir as mybir
    from concourse import bacc
    from concourse.bass import ts

    F32 = mybir.dt.float32
    BF = mybir.dt.bfloat16
    F8 = mybir.dt.float8e4
    DR = mybir.MatmulPerfMode.DoubleRow
    GELU = mybir.ActivationFunctionType.Gelu
    COPY = mybir.ActivationFunctionType.Copy

    nc = bacc.Bacc(None, target_bir_lowering=False)
    xT = nc.declare_dram_parameter("xT", [128, NT, KC, TT], BF, isOutput=False)
    w1a = nc.declare_dram_parameter("w1a", [128, HC, KC, 128], BF, isOutput=False)
    w2s = nc.declare_dram_parameter("w2s", [128, KC, HC, 128], BF, isOutput=False)
    # fp8 pair-interleaved W2 slices (x S2_SCALE): u 0..3 = FP8_PASS's four
    # chunk pairs, u 4..5 = FP8H_PASS's chunk-0..3 pairs.
    # w2f8[p, dc, u, j, d] = W2[e][(hc(u) + j)*128 + p, dc*128+d]*S2
    w2f8 = nc.declare_dram_parameter("w2f8", [128, KC, 6, 2, 128], F8, isOutput=False)
    yT = nc.declare_dram_parameter("yT", [128, NQ_G, KC, M], BF, isOutput=True)

    HQ = 8  # H-chunks (of 128) per pass
    NQ = HC // HQ  # 8 passes; weights stream exactly once
    assert NQ == NQ_G

    with tile.TileContext(nc) as tc:
        with (
            tc.tile_pool(name="xpool", bufs=1) as xpool,
            tc.tile_pool(name="ystage", bufs=6) as ystage,
            tc.tile_pool(name="w1pool", bufs=10) as w1pool,
            tc.tile_pool(name="w2pool", bufs=3) as w2pool,
            tc.tile_pool(name="hpool", bufs=1) as hpool,
            tc.tile_pool(name="psa", bufs=4, space="PSUM") as psa,
            tc.tile_pool(name="psb", bufs=4, space="PSUM") as psb,
        ):
            x_sb = xpool.tile([128, NT, KC, TT], BF, tag="x")
            # Warm the PE HAM clock (cold 1.2GHz -> warm 2.4GHz needs ~3.4us of
            # sustained PE activity) with junk matmuls on a zeroed tile while
            # the startup DMAs are still in flight.
            # Small junk matmuls start the PE busy-window (HAM 1.2->2.4GHz
            # warmup needs ~3.4us) as early as possible: a [128,128] tile
            # memsets in ~150ns (vs 522ns for 512 cols), and N=128 MMs
            # (~107ns cold each) give fine queue granularity so the first
            # real MM dequeues right when its data lands.
            # The first x/w1 chunks only land ~5.6us after init-exit (early
            # DMA is far below peak rate), so the junk-MM bridge must span
            # that whole window or the HAM MID-window (~3.4us idle) drops the
            # PE back to 1.2GHz and the first real MMs run cold (measured
            # 634ns vs 379). 52 x 107ns cold N=128 MMs cover it with fine
            # dequeue granularity.
            # 46 MMs x 107ns cold = ~4.9us, ending right at the measured
            # first-data arrival (~12.6us) — enough to hold the HAM busy
            # window (a sub-3.4us idle gap cannot re-throttle), short enough
            # not to queue ahead of the first real MM.
            warm_sb = xpool.tile([128, 128], BF, tag="warm")
            nc.gpsimd.memset(warm_sb[:], 0.0)
            ps_warm = psa.tile([64, 128], F32, tag="psa")
            fill_state = [0]

            def fillers(n):
                # Junk MMs on a zeroed tile: keep the PE busy (HAM warm
                # grant needs a ~full busy window or it re-throttles to
                # 1.2GHz) while DMA-paced. ~107ns each cold, 54ns warm.
                for k in range(n):
                    nc.tensor.matmul(
                        ps_warm[:],
                        lhsT=warm_sb[:, :64],
                        rhs=warm_sb[:],
                        start=(k == 0),
                        stop=(k == n - 1),
                    )
                fill_state[0] += 1

            fillers(26)
            # Startup DMA order v3. Measured: the two HW-DGE queues (SP +
            # ACT) share one global early-bandwidth ramp (~25 -> 350 B/ns
            # by ~13.5us), so dual queues don't add bandwidth — but
            # splitting the critical set across both halves each queue's
            # backlog so the critical 2.5MB (x[t0] + w1[0]) completes
            # ~16.5us instead of ~20.5us. Interleave by need-order:
            #   SP : x[kc0] x[kc1] x[kc4:8] x[kc12:16]  then x[t1]
            #   ACT: w1[0]q0 x[kc2:4] w1[0]q1 x[kc8:12] w1[0]q2 w1[0]q3
            #        then w1[1..7] whole
            N_EARLY_W1 = 8
            w1_early = [
                w1pool.tile([128, KC, 128], BF, tag="w1", name="w1_early0")
            ]
            nc.scalar.dma_start(w1_early[0][:, ts(0, 4), :], w1a[:, 0, ts(0, 4)])
            nc.sync.dma_start(x_sb[:, 0, 0:1, :], xT[:, 0, 0:1, :])
            nc.sync.dma_start(x_sb[:, 0, 1:2, :], xT[:, 0, 1:2, :])
            nc.scalar.dma_start(x_sb[:, 0, 2:4, :], xT[:, 0, 2:4, :])
            nc.sync.dma_start(x_sb[:, 0, 4:8, :], xT[:, 0, 4:8, :])
            nc.scalar.dma_start(w1_early[0][:, ts(1, 4), :], w1a[:, 0, ts(1, 4)])
            nc.scalar.dma_start(x_sb[:, 0, 8:12, :], xT[:, 0, 8:12, :])
            nc.sync.dma_start(x_sb[:, 0, 12:16, :], xT[:, 0, 12:16, :])
            nc.scalar.dma_start(w1_early[0][:, ts(2, 4), :], w1a[:, 0, ts(2, 4)])
            nc.scalar.dma_start(w1_early[0][:, ts(3, 4), :], w1a[:, 0, ts(3, 4)])
            for i in range(1, N_EARLY_W1):
                w1_sb = w1pool.tile([128, KC, 128], BF, tag="w1", name=f"w1_early{i}")
                nc.scalar.dma_start(w1_sb[:], w1a[:, i])
                w1_early.append(w1_sb)
            for c in range(2):
                nc.sync.dma_start(
                    x_sb[:, 1, ts(c, 8), :], xT[:, 1, ts(c, 8), :]
                )

            for q in range(NQ):
                # Pass FP8_PASS keeps h in e4m3: its phase B runs as fp8
                # DoubleRow matmuls (pairs of adjacent h-chunks), which
                # stream 2 MACs/cell/cycle. Quantizing h + W2 for 1/8 of
                # H adds ~1.3e-2 L2 error (measured on the real seed)
                # on top of the 3.8e-3 bf16 baseline — inside the 2e-2
                # budget. gelu's ACT writes convert PSUM f32 -> fp8.
                if q == FP8H_PASS:
                    hT_sb = hpool.tile([128, HQ // 2, M], BF, tag="h")
                    hT8b = hpool.tile([128, HQ // 2, M], F8, tag="h8")
                else:
                    hT_sb = hpool.tile(
                        [128, HQ, M], F8 if q == FP8_PASS else BF, tag="h"
                    )

                def h_ap(i, t):
                    if q == FP8H_PASS:
                        if i < HQ // 2:
                            return hT8b[:, i, ts(t, TT)]
                        return hT_sb[:, i - HQ // 2, ts(t, TT)]
                    return hT_sb[:, i, ts(t, TT)]

                # Phase A: hT[q] = gelu(W1[:, q]^T @ x), all M tokens.
                # Pass 0 runs t-outer (all t0 groups, then t1) so x[t1] has a
                # half-phase (~28us) of DMA slack. Later passes pair the two
                # token tiles per kc (kc-outer, t-inner into two concurrent
                # PSUM groups) so consecutive MMs share the stationary weights.
                if q == 0:
                    w1_tiles = []
                    for t in range(NT):
                        for i in range(HQ):
                            if t == 0:
                                if i < N_EARLY_W1:
                                    w1_sb = w1_early[i]
                                else:
                                    w1_sb = w1pool.tile(
                                        [128, KC, 128], BF, tag="w1"
                                    )
                                    nc.sync.dma_start(w1_sb[:], w1a[:, i])
                                w1_tiles.append(w1_sb)
                            else:
                                w1_sb = w1_tiles[i]
                            ps = psa.tile([128, TT], F32, tag="psa")
                            # Group 0 trails the x[t0] DMA stream; filler
                            # bursts absorb the predicted inter-chunk waits
                            # so the PE never idles (sized from the traced
                            # arrival curve, cold-rate 107ns each).
                            fill_after = (
                                {1: 8, 3: 16, 7: 14, 11: 8, 15: 8}
                                if (t == 0 and i == 0)
                                else {}
                            )
                            for kc in range(KC):
                                nc.tensor.matmul(
                                    ps[:],
                                    lhsT=w1_sb[:, kc],
                                    rhs=x_sb[:, t, kc, :],
                                    start=(kc == 0),
                                    stop=(kc == KC - 1),
                                )
                                if kc in fill_after:
                                    fillers(fill_after[kc])
                            nc.scalar.activation(h_ap(i, t), ps[:], GELU)
                else:
                    for i in range(HQ):
                        hc = q * HQ + i
                        w1_sb = w1pool.tile([128, KC, 128], BF, tag="w1")
                        nc.sync.dma_start(w1_sb[:], w1a[:, hc])
                        psp = [
                            psa.tile([128, TT], F32, tag="psa", name=f"psA{q}_{i}_{t}")
                            for t in range(NT)
                        ]
                        for kc in range(KC):
                            for t in range(NT):
                                nc.tensor.matmul(
                                    psp[t][:],
                                    lhsT=w1_sb[:, kc],
                                    rhs=x_sb[:, t, kc, :],
                                    start=(kc == 0),
                                    stop=(kc == KC - 1),
                                )
                        for t in range(NT):
                            nc.scalar.activation(h_ap(i, t), psp[t][:], GELU)

                # Phase B: emit partial y_q = W2[q]^T @ hT[q] (bf16) straight
                # to DRAM; the host sums the NQ partials in f32. ACT does the
                # PSUM->SBUF copy (it is idle during phase B), freeing each
                # PSUM bank ~427ns after its group ends — no DVE add chain,
                # no bank-recycle stalls.
                for dc in range(KC):
                    psp2 = [
                        psb.tile([128, TT], F32, tag="psb", name=f"psB{q}_{dc}_{t}")
                        for t in range(NT)
                    ]
                    if q == FP8_PASS:
                        w28_sb = w2pool.tile([128, 4, 2, 128], F8, tag="w28")
                        nc.sync.dma_start(w28_sb[:], w2f8[:, dc, 0:4])
                        for u in range(HQ // 2):
                            for t in range(NT):
                                nc.tensor.matmul(
                                    psp2[t][:],
                                    lhsT=w28_sb[:, u],
                                    rhs=hT_sb[:, 2 * u : 2 * u + 2, ts(t, TT)],
                                    start=(u == 0),
                                    stop=(u == HQ // 2 - 1),
                                    perf_mode=DR,
                                )
                    elif q == FP8H_PASS:
                        # Mixed group: h-chunks 0..3 as 2 fp8 DoubleRow
                        # pair-MMs, chunks 4..7 in bf16. Both operand sets
                        # carry the same x128 W2 prescale so they share
                        # one PSUM accumulation group.
                        w28_sb = w2pool.tile([128, 2, 2, 128], F8, tag="w28")
                        nc.sync.dma_start(w28_sb[:], w2f8[:, dc, 4:6])
                        w2_sb = w2pool.tile([128, HQ // 2, 128], BF, tag="w2")
                        nc.sync.dma_start(
                            w2_sb[:],
                            w2s[:, dc, q * HQ + HQ // 2 : (q + 1) * HQ],
                        )
                        for u in range(2):
                            for t in range(NT):
                                nc.tensor.matmul(
                                    psp2[t][:],
                                    lhsT=w28_sb[:, u],
                                    rhs=hT8b[:, 2 * u : 2 * u + 2, ts(t, TT)],
                                    start=(u == 0),
                                    stop=False,
                                    perf_mode=DR,
                                )
                        for i in range(HQ // 2):
                            for t in range(NT):
                                nc.tensor.matmul(
                                    psp2[t][:],
                                    lhsT=w2_sb[:, i],
                                    rhs=hT_sb[:, i, ts(t, TT)],
                                    start=False,
                                    stop=(i == HQ // 2 - 1),
                                )
                    else:
                        w2_sb = w2pool.tile([128, HQ, 128], BF, tag="w2")
                        nc.sync.dma_start(w2_sb[:], w2s[:, dc, ts(q, HQ)])
                        for i in range(HQ):
                            for t in range(NT):
                                nc.tensor.matmul(
                                    psp2[t][:],
                                    lhsT=w2_sb[:, i],
                                    rhs=hT_sb[:, i, ts(t, TT)],
                                    start=(i == 0),
                                    stop=(i == HQ - 1),
                                )
                    for t in range(NT):
                        yst = ystage.tile([128, TT], BF, tag="yst")
                        nc.scalar.activation(
                            yst[:], psp2[t][:], COPY, scale=1.0 / S2_SCALE
                        )
                        nc.sync.dma_start(yT[:, q, dc, ts(t, TT)], yst[:])
    nc.finalize()
    return nc


def _route(x, Wg, bg):
    """Replicate the reference routing math with jax on CPU.

    Returns (sel_idx, p): [E, CAP] int64 token ids and [E, CAP] f32 weights.
    """
    import jax
    import jax.numpy as jnp

    cpu = jax.devices("cpu")[0]
    with jax.default_device(cpu):
        flat_x = jnp.asarray(x.reshape(N, D))
        logits = flat_x @ jnp.asarray(Wg) + jnp.asarray(bg)
        top_vals, top_idx = jax.lax.top_k(logits, TOPK)
        sparse = jnp.full_like(logits, -jnp.inf)
        sparse = sparse.at[jnp.arange(N)[:, None], top_idx].set(top_vals)
        probs = jax.nn.softmax(sparse, axis=-1)

        sel_idx = np.zeros((E, CAP), dtype=np.int64)
        p_all = np.zeros((E, CAP), dtype=np.float32)
        for i in range(E):
            assigned = (top_idx == i).any(axis=-1)
            score = jnp.where(assigned, probs[:, i], -jnp.inf)
            sel_p, sidx = jax.lax.top_k(score, CAP)
            p = jnp.where(jnp.isfinite(sel_p), sel_p, 0.0)
            sel_idx[i] = np.asarray(sidx)
            p_all[i] = np.asarray(p)
    return sel_idx, p_all


def kernel(x, Wg, bg, W1, W2):
    from concourse.bass_utils import run_bass_kernel_spmd

    x = np.asarray(x, dtype=np.float32)
    W1 = np.asarray(W1, dtype=np.float32)
    W2 = np.asarray(W2, dtype=np.float32)
    sel_idx, p_all = _route(x, np.asarray(Wg, np.float32), np.asarray(bg, np.float32))

    flat_x = x.reshape(N, D)

    # Host dispatch + weight shuffles (bf16).
    w1a = [
        np.ascontiguousarray(
            W1[e].reshape(KC, 128, HC, 128).transpose(1, 2, 0, 3)
        ).astype(BF16)
        for e in range(E)
    ]
    # All of W2 is pre-scaled by S2_SCALE (exact pow2 in bf16); the device
    # undoes it in the PSUM->SBUF copy. This lets fp8 (needs the scale) and
    # bf16 W2 coexist in one accumulation group.
    w2s = [
        np.ascontiguousarray(
            (W2[e] * S2_SCALE).reshape(HC, 128, KC, 128).transpose(1, 2, 0, 3)
        ).astype(BF16)
        for e in range(E)
    ]
    H6 = FP8_PASS * 8 * 128
    H5 = FP8H_PASS * 8 * 128
    w2f8 = []
    for e in range(E):
        rows = np.concatenate(
            [W2[e][H6 : H6 + 1024], W2[e][H5 : H5 + 512]], axis=0
        )  # [1536, D]: u 0..3 = pass-6 pairs, u 4..5 = pass-5 chunk-0..3 pairs
        w2f8.append(
            np.ascontiguousarray(
                (rows * S2_SCALE)
                .reshape(6, 2, 128, KC, 128)
                .transpose(2, 3, 0, 1, 4)
            ).astype(ml_dtypes.float8_e4m3)
        )
    in_maps = []
    for c in range(8):
        e, half = divmod(c, 2)
        tok = flat_x[sel_idx[e, half * M : (half + 1) * M]]  # [M, D]
        # xT[p, t, kc, m] = tok[t*TT+m, kc*128+p]
        xT = np.ascontiguousarray(
            tok.reshape(NT, TT, KC, 128).transpose(3, 0, 2, 1)
        ).astype(BF16)
        in_maps.append({"xT": xT, "w1a": w1a[e], "w2s": w2s[e], "w2f8": w2f8[e]})

    if _nc_cache[0] is None:
        _nc_cache[0] = _build_nc()
    nc = _nc_cache[0]

    trace = bool(os.environ.get("BASS_MOE_TRACE"))
    kwargs = {}
    if trace:
        import sys
        import types

        try:
            from antenv.axon_hooks import get_axon_ntff_profile_hook  # noqa: F401
        except ImportError:
            from trn_agent_boot.trn_boot import _ntff_profile_via_ctypes

            hook = _ntff_profile_via_ctypes("/opt/axon/libaxon_pjrt.so")
            mod = types.ModuleType("antenv.axon_hooks")
            mod.get_axon_ntff_profile_hook = lambda: hook
            import antenv  # noqa: F401

            sys.modules["antenv.axon_hooks"] = mod
        tcores = [int(c) for c in os.environ.get("BASS_MOE_TRACE_CORES", "0").split(",")]
        kwargs = {"trace": True, "trace_cores": tcores}

    res = run_bass_kernel_spmd(nc, in_maps, core_ids=list(range(8)), **kwargs)
    if trace:
        kernel.last_exec_time_ns = res.exec_time_ns
        if res.exec_time_ns is not None:
            print(f"HW exec time: {res.exec_time_ns} ns")

    # Host combine: y = yT^T * p, scatter-add per expert in order.
    out = np.zeros((N, D), dtype=np.float32)
    for c in range(8):
        e, half = divmod(c, 2)
        yq = np.asarray(res.results[c]["yT"], dtype=np.float32)  # [128, NQ, KC, M]
        yT = yq.sum(axis=1)  # [128, KC, M]
        y = yT.transpose(1, 0, 2).reshape(D, M).T  # [M, D]
        p = p_all[e, half * M : (half + 1) * M]
        np.add.at(out, sel_idx[e, half * M : (half + 1) * M], y * p[:, None])
    return out.reshape(B, T, D)



# revision 19
# speedup vs baseline: 1.0474x; 1.0044x over previous
"""MoE block (B=4,T=2048,D=2048,E=4,H=8192,TOPK=2,cap=2048) on 8 TRN2 NeuronCores.

Strategy:
  - Router + top-k + capacity selection on host (exact jax-on-CPU replication of
    the reference routing math, so routing decisions match bit-for-bit).
  - Expert-parallel device FFN: core c handles expert c//2, token half c%2.
    Each core computes yT = W2[e]^T @ gelu(W1[e]^T @ xT) for its 1024 tokens.
    All matmuls in bf16 (same 1 col/cycle PE rate as fp32r but half the DMA,
    FWL-accelerated LDWEIGHTS, and no early HBM starvation), accumulation and
    y output in f32.
  - Host combines: scale by router prob and scatter-add into the output.

Device kernel layout (per core; host pre-transposes so every DMA line is
contiguous per partition):
  xT   [128, NT, KC, TT]   xT[p, t, kc, m]   = tok[t*TT+m, kc*128+p]   bf16
  w1a  [128, 64, 16, 128]  w1a[p, hc, kc, h] = W1[e][kc*128+p, hc*128+h] bf16
  w2s  [128, 16, 64, 128]  w2s[p, dc, hc, d] = W2[e][hc*128+p, dc*128+d] bf16
  yT   [128, 16, 1024]     yT[p, dc, m]      = y[m, dc*128+p]          f32
H is processed in 8 passes of 1024 (8 H-chunks of 128) so each weight byte
streams from HBM exactly once (DMA ~68MB in /16MB out per core).
Per pass: phase A computes hT (bf16) for all 1024 tokens — pass 0 t-outer
(so the second token-tile's x DMA gets ~28us of slack), later passes pair
the two token tiles per kc into two concurrent PSUM groups; phase B (same
pairing) emits a bf16 partial y_q per pass straight to DRAM via an ACT
PSUM->SBUF copy (ACT is idle in phase B; no DVE accumulation chain), and
the host sums the 8 partials in f32 during combine.
Matmuls are [128,128]x[128,512] bf16 at the 216ns/MM streaming floor
(512 cols / 2.4GHz + NX issue; LDWEIGHTS 97ns via FWL, fully hidden).
Measured ~905us/core HW exec (fp32r baseline: 961us) = 97.4% PE-active;
the rest is fixed NEFF init (~7.2us), DMA-bound rampup (~4us), periodic
432ns PE issue hiccups (~5us, hardware), and the epilogue (~5.5us).
Note: the chip sometimes sits in the P0 power state (PE 2.4->2.0GHz), which
inflates any measurement by ~1.2x; gaps then pace at 259ns instead of 216.
"""

import os

import numpy as np
import ml_dtypes

BF16 = ml_dtypes.bfloat16

B, T, D, E, H = 4, 2048, 2048, 4, 8192
TOPK = 2
N = B * T
CAP = N // E          # 2048 tokens per expert
M = CAP // 2          # 1024 tokens per core
KC = D // 128         # 16
HC = H // 128         # 64
TT = 512              # token tile
NT = M // TT          # 2 token tiles
NQ_G = 8              # H passes; device emits one bf16 partial-y per pass
FP8_PASS = 6          # this pass's phase B runs in fp8 DoubleRow (2x pump)
FP8H_PASS = 5         # this pass's phase B is fp8 for h-chunks 0..3 only
S2_SCALE = 128.0      # pow2 prescale on ALL of W2 (undone in the ACT copy);
                      # makes the fp8 slices (needs the scale to clear e4m3
                      # subnormals) and the bf16 rest uniformly scaled so
                      # mixed bf16+fp8 PSUM groups stay consistent

_nc_cache = [None]


def _build_nc():
    import concourse.tile as tile
    import concourse.mybir as mybir
    from concourse import bacc
    from concourse.bass import ts

    F32 = mybir.dt.float32
    BF = mybir.dt.bfloat16
    F8 = mybir.dt.float8e4
    DR = mybir.MatmulPerfMode.DoubleRow
    GELU = mybir.ActivationFunctionType.Gelu
    COPY = mybir.ActivationFunctionType.Copy

    nc = bacc.Bacc(None, target_bir_lowering=False)
    xT = nc.declare_dram_parameter("xT", [128, NT, KC, TT], BF, isOutput=False)
    w1a = nc.declare_dram_parameter("w1a", [128, HC, KC, 128], BF, isOutput=False)
    w2s = nc.declare_dram_parameter("w2s", [128, KC, HC, 128], BF, isOutput=False)
    # fp8 pair-interleaved W2 slices (x S2_SCALE): u 0..3 = FP8_PASS's four
    # chunk pairs, u 4..5 = FP8H_PASS's chunk-0..3 pairs.
    # w2f8[p, dc, u, j, d] = W2[e][(hc(u) + j)*128 + p, dc*128+d]*S2
    w2f8 = nc.declare_dram_parameter("w2f8", [128, KC, 6, 2, 128], F8, isOutput=False)
    yT = nc.declare_dram_parameter("yT", [128, NQ_G, KC, M], BF, isOutput=True)

    HQ = 8  # H-chunks (of 128) per pass
    NQ = HC // HQ  # 8 passes; weights stream exactly once
    assert NQ == NQ_G

    with tile.TileContext(nc) as tc:
        with (
            tc.tile_pool(name="xpool", bufs=1) as xpool,
            tc.tile_pool(name="ystage", bufs=6) as ystage,
            tc.tile_pool(name="w1pool", bufs=10) as w1pool,
            tc.tile_pool(name="w2pool", bufs=3) as w2pool,
            tc.tile_pool(name="hpool", bufs=1) as hpool,
            tc.tile_pool(name="psa", bufs=4, space="PSUM") as psa,
            tc.tile_pool(name="psb", bufs=4, space="PSUM") as psb,
        ):
            x_sb = xpool.tile([128, NT, KC, TT], BF, tag="x")
            # Warm the PE HAM clock (cold 1.2GHz -> warm 2.4GHz needs ~3.4us of
            # sustained PE activity) with junk matmuls on a zeroed tile while
            # the startup DMAs are still in flight.
            # Small junk matmuls start the PE busy-window (HAM 1.2->2.4GHz
            # warmup needs ~3.4us) as early as possible: a [128,128] tile
            # memsets in ~150ns (vs 522ns for 512 cols), and N=128 MMs
            # (~107ns cold each) give fine queue granularity so the first
            # real MM dequeues right when its data lands.
            # The first x/w1 chunks only land ~5.6us after init-exit (early
            # DMA is far below peak rate), so the junk-MM bridge must span
            # that whole window or the HAM MID-window (~3.4us idle) drops the
            # PE back to 1.2GHz and the first real MMs run cold (measured
            # 634ns vs 379). 52 x 107ns cold N=128 MMs cover it with fine
            # dequeue granularity.
            # 46 MMs x 107ns cold = ~4.9us, ending right at the measured
            # first-data arrival (~12.6us) — enough to hold the HAM busy
            # window (a sub-3.4us idle gap cannot re-throttle), short enough
            # not to queue ahead of the first real MM.
            warm_sb = xpool.tile([128, 128], BF, tag="warm")
            nc.gpsimd.memset(warm_sb[:], 0.0)
            ps_warm = psa.tile([64, 128], F32, tag="psa")
            fill_state = [0]

            def fillers(n):
                # Junk MMs on a zeroed tile: keep the PE busy (HAM warm
                # grant needs a ~full busy window or it re-throttles to
                # 1.2GHz) while DMA-paced. ~107ns each cold, 54ns warm.
                for k in range(n):
                    nc.tensor.matmul(
                        ps_warm[:],
                        lhsT=warm_sb[:, :64],
                        rhs=warm_sb[:],
                        start=(k == 0),
                        stop=(k == n - 1),
                    )
                fill_state[0] += 1

            fillers(26)
            # Startup DMA order v3. Measured: the two HW-DGE queues (SP +
            # ACT) share one global early-bandwidth ramp (~25 -> 350 B/ns
            # by ~13.5us), so dual queues don't add bandwidth — but
            # splitting the critical set across both halves each queue's
            # backlog so the critical 2.5MB (x[t0] + w1[0]) completes
            # ~16.5us instead of ~20.5us. Interleave by need-order:
            #   SP : x[kc0] x[kc1] x[kc4:8] x[kc12:16]  then x[t1]
            #   ACT: w1[0]q0 x[kc2:4] w1[0]q1 x[kc8:12] w1[0]q2 w1[0]q3
            #        then w1[1..7] whole
            N_EARLY_W1 = 8
            w1_early = [
                w1pool.tile([128, KC, 128], BF, tag="w1", name=f"w1_early{i}")
                for i in range(N_EARLY_W1)
            ]
            # SP queue: x only, in consumption order. ACT queue: x[4:8] up
            # front (balances the queues' shared ramp), then w1[0] quarters /
            # w1[1] halves sliced to just lead the progressive t0 sweep.
            nc.scalar.dma_start(w1_early[0][:, ts(0, 4), :], w1a[:, 0, ts(0, 4)])
            nc.sync.dma_start(x_sb[:, 0, 0:1, :], xT[:, 0, 0:1, :])
            nc.sync.dma_start(x_sb[:, 0, 1:4, :], xT[:, 0, 1:4, :])
            nc.scalar.dma_start(x_sb[:, 0, 4:8, :], xT[:, 0, 4:8, :])
            nc.scalar.dma_start(w1_early[0][:, ts(1, 4), :], w1a[:, 0, ts(1, 4)])
            nc.scalar.dma_start(w1_early[1][:, 0:8, :], w1a[:, 1, 0:8])
            nc.sync.dma_start(x_sb[:, 0, 8:12, :], xT[:, 0, 8:12, :])
            nc.scalar.dma_start(w1_early[0][:, ts(2, 4), :], w1a[:, 0, ts(2, 4)])
            nc.scalar.dma_start(w1_early[0][:, ts(3, 4), :], w1a[:, 0, ts(3, 4)])
            nc.scalar.dma_start(w1_early[1][:, 8:16, :], w1a[:, 1, 8:16])
            nc.sync.dma_start(x_sb[:, 0, 12:16, :], xT[:, 0, 12:16, :])
            for i in range(2, N_EARLY_W1):
                nc.scalar.dma_start(w1_early[i][:], w1a[:, i])
            for c in range(2):
                nc.sync.dma_start(
                    x_sb[:, 1, ts(c, 8), :], xT[:, 1, ts(c, 8), :]
                )

            for q in range(NQ):
                # Pass FP8_PASS keeps h in e4m3: its phase B runs as fp8
                # DoubleRow matmuls (pairs of adjacent h-chunks), which
                # stream 2 MACs/cell/cycle. Quantizing h + W2 for 1/8 of
                # H adds ~1.3e-2 L2 error (measured on the real seed)
                # on top of the 3.8e-3 bf16 baseline — inside the 2e-2
                # budget. gelu's ACT writes convert PSUM f32 -> fp8.
                if q == FP8H_PASS:
                    hT_sb = hpool.tile([128, HQ // 2, M], BF, tag="h")
                    hT8b = hpool.tile([128, HQ // 2, M], F8, tag="h8")
                else:
                    hT_sb = hpool.tile(
                        [128, HQ, M], F8 if q == FP8_PASS else BF, tag="h"
                    )

                def h_ap(i, t):
                    if q == FP8H_PASS:
                        if i < HQ // 2:
                            return hT8b[:, i, ts(t, TT)]
                        return hT_sb[:, i - HQ // 2, ts(t, TT)]
                    return hT_sb[:, i, ts(t, TT)]

                # Phase A: hT[q] = gelu(W1[:, q]^T @ x), all M tokens.
                # Pass 0 runs t-outer (all t0 groups, then t1) so x[t1] has a
                # half-phase (~28us) of DMA slack. Later passes pair the two
                # token tiles per kc (kc-outer, t-inner into two concurrent
                # PSUM groups) so consecutive MMs share the stationary weights.
                if q == 0:
                    # Progressive t0 sweep: groups 0-1 trail the DMA ramp
                    # kc-by-kc (PE demand 74-150 B/ns matches the early
                    # ramp), groups 2-7 run sequentially once the ramp is
                    # at peak. Filler bursts bridge the predicted early
                    # inter-chunk waits so the HAM warm grant never drops.
                    ps_t0 = {}

                    def t0_mm(i, kc):
                        if i not in ps_t0:
                            ps_t0[i] = psa.tile(
                                [128, TT], F32, tag="psa", name=f"psA0_{i}"
                            )
                        nc.tensor.matmul(
                            ps_t0[i][:],
                            lhsT=w1_early[i][:, kc],
                            rhs=x_sb[:, 0, kc, :],
                            start=(kc == 0),
                            stop=(kc == KC - 1),
                        )
                        if kc == KC - 1:
                            nc.scalar.activation(h_ap(i, 0), ps_t0[i][:], GELU)

                    t0_mm(0, 0)
                    fillers(12)
                    for kc in range(1, 4):
                        t0_mm(0, kc)
                    fillers(4)
                    for kc in range(4, 8):
                        t0_mm(0, kc)
                    for kc in range(0, 8):
                        t0_mm(1, kc)
                    for kc in range(8, 12):
                        t0_mm(0, kc)
                        t0_mm(1, kc)
                    fillers(2)
                    for kc in range(12, 16):
                        t0_mm(0, kc)
                        t0_mm(1, kc)
                    for i in range(2, HQ):
                        for kc in range(KC):
                            t0_mm(i, kc)
                    # t=1: group-outer, reusing the SBUF-resident w1 tiles.
                    for i in range(HQ):
                        ps1 = psa.tile(
                            [128, TT], F32, tag="psa", name=f"psA0t1_{i}"
                        )
                        for kc in range(KC):
                            nc.tensor.matmul(
                                ps1[:],
                                lhsT=w1_early[i][:, kc],
                                rhs=x_sb[:, 1, kc, :],
                                start=(kc == 0),
                                stop=(kc == KC - 1),
                            )
                        nc.scalar.activation(h_ap(i, 1), ps1[:], GELU)
                else:
                    for i in range(HQ):
                        hc = q * HQ + i
                        w1_sb = w1pool.tile([128, KC, 128], BF, tag="w1")
                        nc.sync.dma_start(w1_sb[:], w1a[:, hc])
                        psp = [
                            psa.tile([128, TT], F32, tag="psa", name=f"psA{q}_{i}_{t}")
                            for t in range(NT)
                        ]
                        for kc in range(KC):
                            for t in range(NT):
                                nc.tensor.matmul(
                                    psp[t][:],
                                    lhsT=w1_sb[:, kc],
                                    rhs=x_sb[:, t, kc, :],
                                    start=(kc == 0),
                                    stop=(kc == KC - 1),
                                )
                        for t in range(NT):
                            nc.scalar.activation(h_ap(i, t), psp[t][:], GELU)

                # Phase B: emit partial y_q = W2[q]^T @ hT[q] (bf16) straight
                # to DRAM; the host sums the NQ partials in f32. ACT does the
                # PSUM->SBUF copy (it is idle during phase B), freeing each
                # PSUM bank ~427ns after its group ends — no DVE add chain,
                # no bank-recycle stalls.
                for dc in range(KC):
                    psp2 = [
                        psb.tile([128, TT], F32, tag="psb", name=f"psB{q}_{dc}_{t}")
                        for t in range(NT)
                    ]
                    if q == FP8_PASS:
                        w28_sb = w2pool.tile([128, 4, 2, 128], F8, tag="w28")
                        nc.sync.dma_start(w28_sb[:], w2f8[:, dc, 0:4])
                        for u in range(HQ // 2):
                            for t in range(NT):
                                nc.tensor.matmul(
                                    psp2[t][:],
                                    lhsT=w28_sb[:, u],
                                    rhs=hT_sb[:, 2 * u : 2 * u + 2, ts(t, TT)],
                                    start=(u == 0),
                                    stop=(u == HQ // 2 - 1),
                                    perf_mode=DR,
                                )
                    elif q == FP8H_PASS:
                        # Mixed group: h-chunks 0..3 as 2 fp8 DoubleRow
                        # pair-MMs, chunks 4..7 in bf16. Both operand sets
                        # carry the same x128 W2 prescale so they share
                        # one PSUM accumulation group.
                        w28_sb = w2pool.tile([128, 2, 2, 128], F8, tag="w28")
                        nc.sync.dma_start(w28_sb[:], w2f8[:, dc, 4:6])
                        w2_sb = w2pool.tile([128, HQ // 2, 128], BF, tag="w2")
                        nc.sync.dma_start(
                            w2_sb[:],
                            w2s[:, dc, q * HQ + HQ // 2 : (q + 1) * HQ],
                        )
                        for u in range(2):
                            for t in range(NT):
                                nc.tensor.matmul(
                                    psp2[t][:],
                                    lhsT=w28_sb[:, u],
                                    rhs=hT8b[:, 2 * u : 2 * u + 2, ts(t, TT)],
                                    start=(u == 0),
                                    stop=False,
                                    perf_mode=DR,
                                )
                        for i in range(HQ // 2):
                            for t in range(NT):
                                nc.tensor.matmul(
                                    psp2[t][:],
                                    lhsT=w2_sb[:, i],
                                    rhs=hT_sb[:, i, ts(t, TT)],
                                    start=False,
                                    stop=(i == HQ // 2 - 1),
                                )
                    else:
                        w2_sb = w2pool.tile([128, HQ, 128], BF, tag="w2")
                        nc.sync.dma_start(w2_sb[:], w2s[:, dc, ts(q, HQ)])
                        for i in range(HQ):
                            for t in range(NT):
                                nc.tensor.matmul(
                                    psp2[t][:],
                                    lhsT=w2_sb[:, i],
                                    rhs=hT_sb[:, i, ts(t, TT)],
                                    start=(i == 0),
                                    stop=(i == HQ - 1),
                                )
                    for t in range(NT):
                        yst = ystage.tile([128, TT], BF, tag="yst")
                        nc.scalar.activation(
                            yst[:], psp2[t][:], COPY, scale=1.0 / S2_SCALE
                        )
                        nc.sync.dma_start(yT[:, q, dc, ts(t, TT)], yst[:])
    nc.finalize()
    return nc


def _route(x, Wg, bg):
    """Replicate the reference routing math with jax on CPU.

    Returns (sel_idx, p): [E, CAP] int64 token ids and [E, CAP] f32 weights.
    """
    import jax
    import jax.numpy as jnp

    cpu = jax.devices("cpu")[0]
    with jax.default_device(cpu):
        flat_x = jnp.asarray(x.reshape(N, D))
        logits = flat_x @ jnp.asarray(Wg) + jnp.asarray(bg)
        top_vals, top_idx = jax.lax.top_k(logits, TOPK)
        sparse = jnp.full_like(logits, -jnp.inf)
        sparse = sparse.at[jnp.arange(N)[:, None], top_idx].set(top_vals)
        probs = jax.nn.softmax(sparse, axis=-1)

        sel_idx = np.zeros((E, CAP), dtype=np.int64)
        p_all = np.zeros((E, CAP), dtype=np.float32)
        for i in range(E):
            assigned = (top_idx == i).any(axis=-1)
            score = jnp.where(assigned, probs[:, i], -jnp.inf)
            sel_p, sidx = jax.lax.top_k(score, CAP)
            p = jnp.where(jnp.isfinite(sel_p), sel_p, 0.0)
            sel_idx[i] = np.asarray(sidx)
            p_all[i] = np.asarray(p)
    return sel_idx, p_all


def kernel(x, Wg, bg, W1, W2):
    from concourse.bass_utils import run_bass_kernel_spmd

    x = np.asarray(x, dtype=np.float32)
    W1 = np.asarray(W1, dtype=np.float32)
    W2 = np.asarray(W2, dtype=np.float32)
    sel_idx, p_all = _route(x, np.asarray(Wg, np.float32), np.asarray(bg, np.float32))

    flat_x = x.reshape(N, D)

    # Host dispatch + weight shuffles (bf16).
    w1a = [
        np.ascontiguousarray(
            W1[e].reshape(KC, 128, HC, 128).transpose(1, 2, 0, 3)
        ).astype(BF16)
        for e in range(E)
    ]
    # All of W2 is pre-scaled by S2_SCALE (exact pow2 in bf16); the device
    # undoes it in the PSUM->SBUF copy. This lets fp8 (needs the scale) and
    # bf16 W2 coexist in one accumulation group.
    w2s = [
        np.ascontiguousarray(
            (W2[e] * S2_SCALE).reshape(HC, 128, KC, 128).transpose(1, 2, 0, 3)
        ).astype(BF16)
        for e in range(E)
    ]
    H6 = FP8_PASS * 8 * 128
    H5 = FP8H_PASS * 8 * 128
    w2f8 = []
    for e in range(E):
        rows = np.concatenate(
            [W2[e][H6 : H6 + 1024], W2[e][H5 : H5 + 512]], axis=0
        )  # [1536, D]: u 0..3 = pass-6 pairs, u 4..5 = pass-5 chunk-0..3 pairs
        w2f8.append(
            np.ascontiguousarray(
                (rows * S2_SCALE)
                .reshape(6, 2, 128, KC, 128)
                .transpose(2, 3, 0, 1, 4)
            ).astype(ml_dtypes.float8_e4m3)
        )
    in_maps = []
    for c in range(8):
        e, half = divmod(c, 2)
        tok = flat_x[sel_idx[e, half * M : (half + 1) * M]]  # [M, D]
        # xT[p, t, kc, m] = tok[t*TT+m, kc*128+p]
        xT = np.ascontiguousarray(
            tok.reshape(NT, TT, KC, 128).transpose(3, 0, 2, 1)
        ).astype(BF16)
        in_maps.append({"xT": xT, "w1a": w1a[e], "w2s": w2s[e], "w2f8": w2f8[e]})

    if _nc_cache[0] is None:
        _nc_cache[0] = _build_nc()
    nc = _nc_cache[0]

    trace = bool(os.environ.get("BASS_MOE_TRACE"))
    kwargs = {}
    if trace:
        import sys
        import types

        try:
            from antenv.axon_hooks import get_axon_ntff_profile_hook  # noqa: F401
        except ImportError:
            from trn_agent_boot.trn_boot import _ntff_profile_via_ctypes

            hook = _ntff_profile_via_ctypes("/opt/axon/libaxon_pjrt.so")
            mod = types.ModuleType("antenv.axon_hooks")
            mod.get_axon_ntff_profile_hook = lambda: hook
            import antenv  # noqa: F401

            sys.modules["antenv.axon_hooks"] = mod
        tcores = [int(c) for c in os.environ.get("BASS_MOE_TRACE_CORES", "0").split(",")]
        kwargs = {"trace": True, "trace_cores": tcores}

    res = run_bass_kernel_spmd(nc, in_maps, core_ids=list(range(8)), **kwargs)
    if trace:
        kernel.last_exec_time_ns = res.exec_time_ns
        if res.exec_time_ns is not None:
            print(f"HW exec time: {res.exec_time_ns} ns")

    # Host combine: y = yT^T * p, scatter-add per expert in order.
    out = np.zeros((N, D), dtype=np.float32)
    for c in range(8):
        e, half = divmod(c, 2)
        yq = np.asarray(res.results[c]["yT"], dtype=np.float32)  # [128, NQ, KC, M]
        yT = yq.sum(axis=1)  # [128, KC, M]
        y = yT.transpose(1, 0, 2).reshape(D, M).T  # [M, D]
        p = p_all[e, half * M : (half + 1) * M]
        np.add.at(out, sel_idx[e, half * M : (half + 1) * M], y * p[:, None])
    return out.reshape(B, T, D)

